# revision 5
# baseline (speedup 1.0000x reference)
"""DynamicSparseMoE Trainium2 kernel (v2: gate-first, channel-major output).

Math (per token t):
  logits[e'] = x[t] . gate_w[e'] + gate_b[e']        (C=2048 contraction)
  gw[e']     = 1.0 if logits[e'] > 0 else 0.0
  expert e input: xe[d] = x[t, 16*d + e]  (d=0..127; expert idx fastest in channel)
  h  = gelu(fc_w[e] @ xe + fc_b[e])                   (H=512)
  oe = proj_w[e] @ h + proj_b[e]                      (DE=128)
  out[t, 128*e + d] = gw[e] * oe[d]                   (expert-major output channels)

Strategy: data-parallel over the 16384 tokens across 8 NeuronCores (2048
tokens/core).  Per 512-token group, two passes:
  pass A (per expert): 4 PE transposes of the stride-16 channel slice of the
    row-major x tile -> xe^T [de, tok] fp32 (DVE evacuation, kept in SBUF for
    the whole group); exact-fp32 gate matmuls accumulated into one PSUM bank
    (pipelined 2 experts behind the transposes).  Then DVE threshold
    (logits > -gate_b) -> gw [16, tok].
  pass B (per expert): GPSIMD broadcasts gw row e across 128 partitions;
    fc as fp32r matmuls (weights stationary, N=512) on bitcast views of the
    fp32 xe; gelu+fc_bias fused on ACT writing fp32r; proj as fp32r matmuls
    accumulating K=512; single fused DVE scalar_tensor_tensor evacuation
    out^T = (psum + proj_b) * gw_broadcast.
  Output is written CHANNEL-major ([C, tokens]) to DRAM; the host transposes
  during unsharding.  This removes all exit transposes and extra evac work.
"""

import sys

for _p in ("/opt/trn_rl_repo", "/root/.axon_site"):
    if _p not in sys.path:
        sys.path.insert(0, _p)

import ml_dtypes
import numpy as np

import concourse.mybir as mybir
from concourse import bacc
from concourse.bass_utils import run_bass_kernel_spmd
from concourse.tile import TileContext


B, T, C, E = 8, 2048, 2048, 16
DE = C // E  # 128
H = 4 * DE  # 512
NCORES = 8
NTOK = B * T  # 16384
TPC = NTOK // NCORES  # tokens per core: 2048
GROUP = 512  # tokens per group
NTAU = GROUP // 128  # 4 token-tiles per group
NGRP = TPC // GROUP  # 4 groups per core

F32 = mybir.dt.float32
F32R = mybir.dt.float32r
BF16 = mybir.dt.bfloat16
AF = mybir.ActivationFunctionType
ALU = mybir.AluOpType
GELU = AF.Gelu

_CACHE = {}


def _build():
    nc = bacc.Bacc(trn_type="TRN2", num_devices=NCORES)

    x_d = nc.dram_tensor("x", [TPC, C], F32, kind="ExternalInput").ap()
    gwp_d = nc.dram_tensor("gwp", [C, E], F32, kind="ExternalInput").ap()
    fcw_d = nc.dram_tensor("fcw", [E, DE, H], BF16, kind="ExternalInput").ap()
    pjw_d = nc.dram_tensor("pjw", [E, 4, 128, DE], BF16, kind="ExternalInput").ap()
    fcb_d = nc.dram_tensor("fcb", [128, 64], F32, kind="ExternalInput").ap()
    pjb_d = nc.dram_tensor("pjb", [128, E], F32, kind="ExternalInput").ap()
    ngb_d = nc.dram_tensor("ngb", [16, 1], F32, kind="ExternalInput").ap()
    idn_d = nc.dram_tensor("idn", [128, 128], F32, kind="ExternalInput").ap()
    out_d = nc.dram_tensor("out", [C, TPC], F32, kind="ExternalOutput").ap()

    with TileContext(nc) as tc:
        with (
            tc.tile_pool(name="wts", bufs=1) as wts,
            tc.tile_pool(name="work", bufs=2) as work,
            tc.tile_pool(name="psum", bufs=2, space="PSUM") as psum,
        ):
            # ---- resident weights (plain DMA; fp32r is bitwise fp32) ----
            idn_sb = wts.tile([128, 128], F32)
            nc.sync.dma_start(out=idn_sb, in_=idn_d)
            gwp_sb = wts.tile([128, E * E], F32)  # [d, chunk*16+e']
            nc.sync.dma_start(
                out=gwp_sb.rearrange("p (k e) -> p k e", k=E),
                in_=gwp_d.rearrange("(k p) e -> p k e", p=128),
            )
            ngb_sb = wts.tile([16, 1], F32)
            nc.sync.dma_start(out=ngb_sb, in_=ngb_d)
            fcb_sb = wts.tile([128, 64], F32)
            nc.scalar.dma_start(out=fcb_sb, in_=fcb_d)
            pjb_sb = wts.tile([128, E], F32)
            nc.scalar.dma_start(out=pjb_sb, in_=pjb_d)
            # fc/proj weights: per-expert slices so pass B can start early
            fcw_sb = wts.tile([128, E * H], BF16)  # [de, e*512+h]
            pjw_sb = wts.tile([128, E * 4 * DE], BF16)  # [h_in_chunk, (e*4+q)*128+d]
            pjw_v = pjw_sb.rearrange("p (e q d) -> p e q d", e=E, q=4)
            pjw_dv = pjw_d.rearrange("e q p d -> p e q d")
            for e in range(E):
                eng = nc.scalar if (e % 2) else nc.sync
                eng.dma_start(out=fcw_sb[:, e * H : (e + 1) * H], in_=fcw_d[e])
                eng.dma_start(out=pjw_v[:, e], in_=pjw_dv[:, e])

            for g in range(NGRP):
                t0 = g * GROUP
                xrow = []
                for ti in range(NTAU):
                    xt = work.tile([128, C], F32, tag="xrow", bufs=5)
                    nc.sync.dma_start(
                        out=xt, in_=x_d[t0 + ti * 128 : t0 + (ti + 1) * 128, :]
                    )
                    xrow.append(xt)

                ps_g = psum.tile([16, GROUP], F32, tag="gate", bufs=1)
                xe_all = []
                xb_all = []
                # ---- pass A: transposes + exact fp32 gate (2-expert lag) ----
                for e in range(E):
                    ps_t = psum.tile([128, GROUP], F32, tag="tp", bufs=3)
                    for ti in range(NTAU):
                        lhs = xrow[ti].rearrange("p (d e) -> p e d", e=E)[:, e, :]
                        nc.tensor.transpose(
                            ps_t[:, ti * 128 : (ti + 1) * 128], lhs, idn_sb
                        )
                    xe = work.tile([128, GROUP], F32, tag="xe", bufs=4)
                    nc.vector.tensor_copy(xe, ps_t)
                    xe_all.append(xe)
                    xb = work.tile([128, GROUP], BF16, tag="xb", bufs=18)
                    nc.gpsimd.tensor_copy(xb, xe)
                    xb_all.append(xb)
                    if e >= 2:
                        ep = e - 2
                        nc.tensor.matmul(
                            ps_g,
                            lhsT=gwp_sb[:, ep * E : (ep + 1) * E],
                            rhs=xe_all[ep],
                            start=(ep == 0),
                            stop=False,
                        )
                for ep in (E - 2, E - 1):
                    nc.tensor.matmul(
                        ps_g,
                        lhsT=gwp_sb[:, ep * E : (ep + 1) * E],
                        rhs=xe_all[ep],
                        start=False,
                        stop=(ep == E - 1),
                    )
                # threshold: gw[e', t] = (logits0 > -gate_b[e'])
                gwt = work.tile([16, GROUP], BF16, tag="gwt", bufs=2)
                nc.vector.tensor_scalar(
                    gwt, ps_g, ngb_sb, None, op0=ALU.is_gt
                )
                # flatten the 16 gate rows onto partition 0, then broadcast
                # across all 128 partitions in 4-expert chunks (gpsimd)
                gwf = work.tile([1, E * GROUP], BF16, tag="gwf", bufs=2)
                nc.sync.dma_start(
                    out=gwf.rearrange("o (e t) -> o e t", e=E), in_=gwt
                )
                gwb = work.tile([128, E * GROUP], BF16, tag="gwb", bufs=2)
                for q in range(4):
                    nc.gpsimd.partition_broadcast(
                        gwb[:, q * 4 * GROUP : (q + 1) * 4 * GROUP],
                        gwf[:, q * 4 * GROUP : (q + 1) * 4 * GROUP],
                    )

                # ---- pass B: fc -> gelu -> proj -> fused gated evacuation ----
                outT = None
                for e in range(E):
                    xer = xb_all[e]
                    h_sb = work.tile([128, 4 * GROUP], BF16, tag="h", bufs=2)
                    for hq in range(4):
                        ps_fc = psum.tile([128, GROUP], F32, tag="fc", bufs=2)
                        nc.tensor.matmul(
                            ps_fc,
                            lhsT=fcw_sb[
                                :, e * H + hq * 128 : e * H + (hq + 1) * 128
                            ],
                            rhs=xer,
                            start=True,
                            stop=True,
                        )
                        nc.scalar.activation(
                            h_sb[:, hq * GROUP : (hq + 1) * GROUP],
                            ps_fc,
                            GELU,
                            bias=fcb_sb[:, e * 4 + hq : e * 4 + hq + 1],
                            scale=1.0,
                        )
                    ps_pj = psum.tile([128, GROUP], F32, tag="pj", bufs=2)
                    for hq in range(4):
                        nc.tensor.matmul(
                            ps_pj,
                            lhsT=pjw_sb[
                                :, (e * 4 + hq) * 128 : (e * 4 + hq + 1) * 128
                            ],
                            rhs=h_sb[:, hq * GROUP : (hq + 1) * GROUP],
                            start=(hq == 0),
                            stop=(hq == 3),
                        )
                    if e % 4 == 0:
                        outT = work.tile([128, 4 * GROUP], F32, tag="outT", bufs=2)
                    # out^T[d, t] = (psum + proj_b[d]) * gw[e, t]
                    nc.vector.scalar_tensor_tensor(
                        outT[:, (e % 4) * GROUP : (e % 4 + 1) * GROUP],
                        ps_pj,
                        pjb_sb[:, e : e + 1],
                        gwb[:, e * GROUP : (e + 1) * GROUP],
                        op0=ALU.add,
                        op1=ALU.mult,
                    )
                    if e % 4 == 3:
                        nc.sync.dma_start(
                            out=out_d[
                                (e - 3) * 128 : (e + 1) * 128, t0 : t0 + GROUP
                            ].rearrange("(q p) t -> p q t", p=128),
                            in_=outT.rearrange("p (q t) -> p q t", q=4),
                        )

    nc.compile()
    return nc


def _prep_inputs(x, gate_w, gate_b, fc_w, fc_b, proj_w, proj_b):
    x = np.ascontiguousarray(np.asarray(x, dtype=np.float32)).reshape(NTOK, C)
    gate_w = np.asarray(gate_w, dtype=np.float32)
    gate_b = np.asarray(gate_b, dtype=np.float32)
    fc_w = np.asarray(fc_w, dtype=np.float32)
    fc_b = np.asarray(fc_b, dtype=np.float32)
    proj_w = np.asarray(proj_w, dtype=np.float32)
    proj_b = np.asarray(proj_b, dtype=np.float32)

    # permuted channel order: c' = e*128 + d  ->  orig c = 16*d + e
    cp = np.arange(C)
    orig = 16 * (cp % DE) + cp // DE
    gwp = np.ascontiguousarray(gate_w[:, orig].T)  # [C, E]
    fcw = np.ascontiguousarray(fc_w.transpose(0, 2, 1)).astype(ml_dtypes.bfloat16)
    pjw = np.ascontiguousarray(
        proj_w.transpose(0, 2, 1).reshape(E, 4, 128, DE)
    ).astype(ml_dtypes.bfloat16)  # [E, q, h_in_chunk, d]
    fcb = np.ascontiguousarray(
        fc_b.reshape(E, 4, 128).transpose(2, 0, 1).reshape(128, E * 4)
    )
    pjb = np.ascontiguousarray(proj_b.T)  # [DE, E]
    ngb = np.ascontiguousarray((-gate_b).reshape(16, 1))
    idn = np.eye(128, dtype=np.float32)

    shared = {
        "gwp": gwp,
        "fcw": fcw,
        "pjw": pjw,
        "fcb": fcb,
        "pjb": pjb,
        "ngb": ngb,
        "idn": idn,
    }
    in_maps = [
        {"x": np.ascontiguousarray(x[i * TPC : (i + 1) * TPC]), **shared}
        for i in range(NCORES)
    ]
    return in_maps


def kernel(x, gate_w, gate_b, fc_w, fc_b, proj_w, proj_b, _trace=False, _tmpdir=None):
    if "nc" not in _CACHE:
        _CACHE["nc"] = _build()
    nc = _CACHE["nc"]
    in_maps = _prep_inputs(x, gate_w, gate_b, fc_w, fc_b, proj_w, proj_b)
    res = run_bass_kernel_spmd(
        nc,
        in_maps,
        core_ids=list(range(NCORES)),
        trace=_trace,
        tmpdir=_tmpdir,
    )
    # per-core output is channel-major [C, TPC]; transpose while unsharding
    out = np.concatenate(
        [np.ascontiguousarray(res.results[i]["out"].T) for i in range(NCORES)],
        axis=0,
    )
    out = out.reshape(B, T, C)
    if _trace:
        _CACHE["last_result"] = res
    return out


# revision 12
# speedup vs baseline: 1.2233x; 1.2233x over previous
"""DynamicSparseMoE Trainium2 kernel (v3: DMA-transposed bf16 planes).

Math (per token t):
  logits[e'] = x[t] . gate_w[e'] + gate_b[e']        (C=2048 contraction)
  gw[e']     = 1.0 if logits[e'] > 0 else 0.0
  expert e input: xe[d] = x[t, 16*d + e]  (d=0..127; expert idx fastest in channel)
  h  = gelu(fc_w[e] @ xe + fc_b[e])                   (H=512)
  oe = proj_w[e] @ h + proj_b[e]                      (DE=128)
  out[t, 128*e + d] = gw[e] * oe[d]                   (expert-major output channels)

Strategy: data-parallel over the 16384 tokens across 8 NeuronCores (2048
tokens/core).  The host pre-permutes x to (expert, d)-channel-major order and
splits it into bf16 hi/lo planes (x = xh + xl exactly to ~2^-17).  Per
512-token group:
  - one DMA-XBAR transpose per plane loads all 16 expert tiles [d, tok]
    directly into SBUF (no PE transposes, no PSUM evacuations).
  - gate: 3-term bf16 accumulation xh@gh + xl@gh + xh@gl into one PSUM bank
    (~1e-5 logit accuracy, exact thresholding); DVE threshold vs -gate_b;
    flatten to partition 0 (DMA) and GPSIMD-broadcast to [128, 16*tok].
  - per expert: fc as bf16 matmuls (weights stationary, N=512) on the hi
    plane; gelu fused on ACT (batched over 2 PSUM banks when fc_b == 0);
    proj as bf16 matmuls accumulating K=512; single fused DVE
    scalar_tensor_tensor evacuation out^T = (psum + proj_b) * gw_bcast, bf16.
  Output is CHANNEL-major ([C, tokens]) bf16 in DRAM; the host transposes and
  upcasts while unsharding.
"""

import sys

for _p in ("/opt/trn_rl_repo", "/root/.axon_site"):
    if _p not in sys.path:
        sys.path.insert(0, _p)

import ml_dtypes
import numpy as np

import concourse.mybir as mybir
from concourse import bacc
from concourse.bass_utils import run_bass_kernel_spmd
from concourse.tile import TileContext


B, T, C, E = 8, 2048, 2048, 16
DE = C // E  # 128
H = 4 * DE  # 512
NCORES = 8
NTOK = B * T  # 16384
TPC = NTOK // NCORES  # tokens per core: 2048
GROUP = 512  # tokens per group
NGRP = TPC // GROUP  # 4 groups per core

F32 = mybir.dt.float32
BF16 = mybir.dt.bfloat16
AF = mybir.ActivationFunctionType
ALU = mybir.AluOpType
GELU = AF.Gelu

_CACHE = {}


def _build(split_gelu_bias: bool):
    nc = bacc.Bacc(trn_type="TRN2", num_devices=NCORES)

    xh_d = nc.dram_tensor("xh", [C, TPC], BF16, kind="ExternalInput").ap()
    xl_d = nc.dram_tensor("xl", [C, TPC], BF16, kind="ExternalInput").ap()
    gwh_d = nc.dram_tensor("gwh", [C, E], BF16, kind="ExternalInput").ap()
    gwl_d = nc.dram_tensor("gwl", [C, E], BF16, kind="ExternalInput").ap()
    fcw_d = nc.dram_tensor("fcw", [E, DE, H], BF16, kind="ExternalInput").ap()
    pjw_d = nc.dram_tensor("pjw", [E, 4, 128, DE], BF16, kind="ExternalInput").ap()
    fcb_d = nc.dram_tensor("fcb", [128, 64], F32, kind="ExternalInput").ap()
    pjb_d = nc.dram_tensor("pjb", [128, E], F32, kind="ExternalInput").ap()
    ngb_d = nc.dram_tensor("ngb", [16, 1], F32, kind="ExternalInput").ap()
    out_d = nc.dram_tensor("out", [C, TPC], BF16, kind="ExternalOutput").ap()

    with TileContext(nc) as tc:
        with (
            tc.tile_pool(name="wts", bufs=1) as wts,
            tc.tile_pool(name="work", bufs=2) as work,
            tc.tile_pool(name="psum", bufs=2, space="PSUM") as psum,
        ):
            # ---- group-0 x planes first so the gate starts early ----
            xh_t = [None] * NGRP
            xl_t = [None] * NGRP

            def load_planes(g):
                t0 = g * GROUP
                xh_t[g] = work.tile(
                    [128, E * GROUP], BF16, tag="xh", bufs=2, name=f"xh_{g}"
                )
                nc.sync.dma_start(
                    out=xh_t[g].rearrange("p (k t) -> p k t", k=E),
                    in_=xh_d[:, t0 : t0 + GROUP].rearrange("(k p) t -> p k t", p=128),
                )
                xl_t[g] = work.tile(
                    [128, E * GROUP], BF16, tag="xl", bufs=2, name=f"xl_{g}"
                )
                nc.scalar.dma_start(
                    out=xl_t[g].rearrange("p (k t) -> p k t", k=E),
                    in_=xl_d[:, t0 : t0 + GROUP].rearrange("(k p) t -> p k t", p=128),
                )

            load_planes(0)

            # ---- resident weights ----
            gwh_sb = wts.tile([128, E * E], BF16)  # [d, chunk*16+e']
            nc.sync.dma_start(
                out=gwh_sb.rearrange("p (k e) -> p k e", k=E),
                in_=gwh_d.rearrange("(k p) e -> p k e", p=128),
            )
            gwl_sb = wts.tile([128, E * E], BF16)
            nc.sync.dma_start(
                out=gwl_sb.rearrange("p (k e) -> p k e", k=E),
                in_=gwl_d.rearrange("(k p) e -> p k e", p=128),
            )
            ngb_sb = wts.tile([16, 1], F32)
            nc.sync.dma_start(out=ngb_sb, in_=ngb_d)
            fcb_sb = wts.tile([128, 64], F32)
            nc.scalar.dma_start(out=fcb_sb, in_=fcb_d)
            pjb_sb = wts.tile([128, E], F32)
            nc.scalar.dma_start(out=pjb_sb, in_=pjb_d)
            # fc/proj weights: per-expert slices so pass B can start early
            fcw_sb = wts.tile([128, E * H], BF16)  # [de, e*512+h]
            pjw_sb = wts.tile([128, E * 4 * DE], BF16)  # [h_chunk, (e*4+q)*128+d]
            pjw_v = pjw_sb.rearrange("p (e q d) -> p e q d", e=E, q=4)
            pjw_dv = pjw_d.rearrange("e q p d -> p e q d")
            for e in range(E):
                eng = nc.scalar if (e % 2) else nc.sync
                eng.dma_start(out=fcw_sb[:, e * H : (e + 1) * H], in_=fcw_d[e])
                eng.dma_start(out=pjw_v[:, e], in_=pjw_dv[:, e])

            for g in range(NGRP):
                t0 = g * GROUP
                if g + 1 < NGRP:
                    load_planes(g + 1)
                xh, xl = xh_t[g], xl_t[g]

                # ---- gate: 3-term bf16 accumulation, exact to ~1e-5 ----
                ps_g = psum.tile([16, GROUP], F32, tag="gate", bufs=2)
                for e in range(E):
                    sh = slice(e * GROUP, (e + 1) * GROUP)
                    sw = slice(e * E, (e + 1) * E)
                    nc.tensor.matmul(
                        ps_g, lhsT=gwh_sb[:, sw], rhs=xh[:, sh],
                        start=(e == 0), stop=False,
                    )
                    nc.tensor.matmul(
                        ps_g, lhsT=gwh_sb[:, sw], rhs=xl[:, sh],
                        start=False, stop=False,
                    )
                    nc.tensor.matmul(
                        ps_g, lhsT=gwl_sb[:, sw], rhs=xh[:, sh],
                        start=False, stop=(e == E - 1),
                    )
                # threshold: gw[e', t] = (logits0 > -gate_b[e'])
                gwt = work.tile([16, GROUP], BF16, tag="gwt", bufs=2)
                nc.vector.tensor_scalar(gwt, ps_g, ngb_sb, None, op0=ALU.is_gt)
                # flatten onto partition 0, broadcast across partitions
                gwf = work.tile([1, E * GROUP], BF16, tag="gwf", bufs=2)
                nc.sync.dma_start(
                    out=gwf.rearrange("o (e t) -> o e t", e=E), in_=gwt
                )
                gwb = work.tile([128, E * GROUP], BF16, tag="gwb", bufs=2)
                for q in range(4):
                    nc.gpsimd.partition_broadcast(
                        gwb[:, q * 4 * GROUP : (q + 1) * 4 * GROUP],
                        gwf[:, q * 4 * GROUP : (q + 1) * 4 * GROUP],
                    )

                # ---- per expert: fc -> gelu -> proj -> fused gated evac ----
                outT = None
                for e in range(E):
                    xslab = xh[:, e * GROUP : (e + 1) * GROUP]
                    h_sb = work.tile([128, 4 * GROUP], BF16, tag="h", bufs=2)
                    if split_gelu_bias:
                        for hq in range(4):
                            ps_fc = psum.tile(
                                [128, GROUP], F32, tag="fc", bufs=4
                            )
                            nc.tensor.matmul(
                                ps_fc,
                                lhsT=fcw_sb[
                                    :, e * H + hq * 128 : e * H + (hq + 1) * 128
                                ],
                                rhs=xslab,
                                start=True, stop=True,
                            )
                            nc.scalar.activation(
                                h_sb[:, hq * GROUP : (hq + 1) * GROUP],
                                ps_fc, GELU,
                                bias=fcb_sb[:, e * 4 + hq : e * 4 + hq + 1],
                                scale=1.0,
                            )
                    else:
                        for hp in range(2):
                            ps_fc = psum.tile(
                                [128, 2 * GROUP], F32, tag="fc", bufs=2
                            )
                            for hq in (2 * hp, 2 * hp + 1):
                                nc.tensor.matmul(
                                    ps_fc[:, (hq % 2) * GROUP : (hq % 2 + 1) * GROUP],
                                    lhsT=fcw_sb[
                                        :, e * H + hq * 128 : e * H + (hq + 1) * 128
                                    ],
                                    rhs=xslab,
                                    start=True, stop=True,
                                )
                            nc.scalar.activation(
                                h_sb[:, 2 * hp * GROUP : (2 * hp + 2) * GROUP],
                                ps_fc, GELU, scale=1.0,
                            )
                    ps_pj = psum.tile([128, GROUP], F32, tag="pj", bufs=2)
                    for hq in range(4):
                        nc.tensor.matmul(
                            ps_pj,
                            lhsT=pjw_sb[
                                :, (e * 4 + hq) * 128 : (e * 4 + hq + 1) * 128
                            ],
                            rhs=h_sb[:, hq * GROUP : (hq + 1) * GROUP],
                            start=(hq == 0), stop=(hq == 3),
                        )
                    if e % 4 == 0:
                        outT = work.tile([128, 4 * GROUP], BF16, tag="outT", bufs=3)
                    # out^T[d, t] = (psum + proj_b[d]) * gw[e, t]
                    nc.vector.scalar_tensor_tensor(
                        outT[:, (e % 4) * GROUP : (e % 4 + 1) * GROUP],
                        ps_pj,
                        pjb_sb[:, e : e + 1],
                        gwb[:, e * GROUP : (e + 1) * GROUP],
                        op0=ALU.add,
                        op1=ALU.mult,
                    )
                    if e % 4 == 3:
                        nc.sync.dma_start(
                            out=out_d[
                                (e - 3) * 128 : (e + 1) * 128, t0 : t0 + GROUP
                            ].rearrange("(q p) t -> p q t", p=128),
                            in_=outT.rearrange("p (q t) -> p q t", q=4),
                        )

    nc.compile()
    return nc


def _prep_inputs(x, gate_w, gate_b, fc_w, fc_b, proj_w, proj_b):
    bf16 = ml_dtypes.bfloat16
    x = np.ascontiguousarray(np.asarray(x, dtype=np.float32)).reshape(NTOK, C)
    gate_w = np.asarray(gate_w, dtype=np.float32)
    gate_b = np.asarray(gate_b, dtype=np.float32)
    fc_w = np.asarray(fc_w, dtype=np.float32)
    fc_b = np.asarray(fc_b, dtype=np.float32)
    proj_w = np.asarray(proj_w, dtype=np.float32)
    proj_b = np.asarray(proj_b, dtype=np.float32)

    # permuted channel order: c' = e*128 + d  ->  orig c = 16*d + e
    cp = np.arange(C)
    orig = 16 * (cp % DE) + cp // DE
    xp = x[:, orig]  # (e, d)-channel-major
    xh = xp.astype(bf16)
    xl = (xp - xh.astype(np.float32)).astype(bf16)
    # host-side transpose: ship channel-major planes so the device loads
    # expert slabs with plain contiguous-row DMAs (no on-device transpose)
    xh = np.ascontiguousarray(xh.reshape(NCORES, TPC, C).transpose(0, 2, 1))
    xl = np.ascontiguousarray(xl.reshape(NCORES, TPC, C).transpose(0, 2, 1))
    gwp = gate_w[:, orig].T  # [C', E]
    gwh = gwp.astype(bf16)
    gwl = (gwp - gwh.astype(np.float32)).astype(bf16)
    fcw = np.ascontiguousarray(fc_w.transpose(0, 2, 1)).astype(bf16)  # [E, DE, H]
    pjw = np.ascontiguousarray(
        proj_w.transpose(0, 2, 1).reshape(E, 4, 128, DE)
    ).astype(bf16)  # [E, q, h_chunk, d]
    fcb = np.ascontiguousarray(
        fc_b.reshape(E, 4, 128).transpose(2, 0, 1).reshape(128, E * 4)
    )
    pjb = np.ascontiguousarray(proj_b.T)  # [DE, E]
    ngb = np.ascontiguousarray((-gate_b).reshape(16, 1))

    shared = {
        "gwh": np.ascontiguousarray(gwh),
        "gwl": np.ascontiguousarray(gwl),
        "fcw": fcw,
        "pjw": pjw,
        "fcb": fcb,
        "pjb": pjb,
        "ngb": ngb,
    }
    in_maps = [
        {"xh": xh[i], "xl": xl[i], **shared}
        for i in range(NCORES)
    ]
    return in_maps, bool(np.any(fc_b))


def kernel(x, gate_w, gate_b, fc_w, fc_b, proj_w, proj_b, _trace=False, _tmpdir=None):
    in_maps, split_bias = _prep_inputs(
        x, gate_w, gate_b, fc_w, fc_b, proj_w, proj_b
    )
    key = ("nc", split_bias)
    if key not in _CACHE:
        _CACHE[key] = _build(split_bias)
    nc = _CACHE[key]
    res = run_bass_kernel_spmd(
        nc,
        in_maps,
        core_ids=list(range(NCORES)),
        trace=_trace,
        tmpdir=_tmpdir,
    )
    # per-core output is channel-major bf16 [C, TPC]; transpose + upcast
    out = np.concatenate(
        [
            np.ascontiguousarray(res.results[i]["out"].T.astype(np.float32))
            for i in range(NCORES)
        ],
        axis=0,
    )
    out = out.reshape(B, T, C)
    if _trace:
        _CACHE["last_result"] = res
    return out


# revision 13
# speedup vs baseline: 1.2502x; 1.0220x over previous
"""DynamicSparseMoE Trainium2 kernel (v3: DMA-transposed bf16 planes).

Math (per token t):
  logits[e'] = x[t] . gate_w[e'] + gate_b[e']        (C=2048 contraction)
  gw[e']     = 1.0 if logits[e'] > 0 else 0.0
  expert e input: xe[d] = x[t, 16*d + e]  (d=0..127; expert idx fastest in channel)
  h  = gelu(fc_w[e] @ xe + fc_b[e])                   (H=512)
  oe = proj_w[e] @ h + proj_b[e]                      (DE=128)
  out[t, 128*e + d] = gw[e] * oe[d]                   (expert-major output channels)

Strategy: data-parallel over the 16384 tokens across 8 NeuronCores (2048
tokens/core).  The host pre-permutes x to (expert, d)-channel-major order and
splits it into bf16 hi/lo planes (x = xh + xl exactly to ~2^-17).  Per
512-token group:
  - one DMA-XBAR transpose per plane loads all 16 expert tiles [d, tok]
    directly into SBUF (no PE transposes, no PSUM evacuations).
  - gate: 3-term bf16 accumulation xh@gh + xl@gh + xh@gl into one PSUM bank
    (~1e-5 logit accuracy, exact thresholding); DVE threshold vs -gate_b;
    flatten to partition 0 (DMA) and GPSIMD-broadcast to [128, 16*tok].
  - per expert: fc as bf16 matmuls (weights stationary, N=512) on the hi
    plane; gelu fused on ACT (batched over 2 PSUM banks when fc_b == 0);
    proj as bf16 matmuls accumulating K=512; single fused DVE
    scalar_tensor_tensor evacuation out^T = (psum + proj_b) * gw_bcast, bf16.
  Output is CHANNEL-major ([C, tokens]) bf16 in DRAM; the host transposes and
  upcasts while unsharding.
"""

import sys

for _p in ("/opt/trn_rl_repo", "/root/.axon_site"):
    if _p not in sys.path:
        sys.path.insert(0, _p)

import ml_dtypes
import numpy as np

import concourse.mybir as mybir
from concourse import bacc
from concourse.bass_utils import run_bass_kernel_spmd
from concourse.tile import TileContext


B, T, C, E = 8, 2048, 2048, 16
DE = C // E  # 128
H = 4 * DE  # 512
NCORES = 8
NTOK = B * T  # 16384
TPC = NTOK // NCORES  # tokens per core: 2048
GROUP = 512  # tokens per group
NGRP = TPC // GROUP  # 4 groups per core

F32 = mybir.dt.float32
BF16 = mybir.dt.bfloat16
AF = mybir.ActivationFunctionType
ALU = mybir.AluOpType
GELU = AF.Gelu

_CACHE = {}


def _build(split_gelu_bias: bool):
    nc = bacc.Bacc(trn_type="TRN2", num_devices=NCORES)

    xh_d = nc.dram_tensor("xh", [C, TPC], BF16, kind="ExternalInput").ap()
    xl_d = nc.dram_tensor("xl", [C, TPC], BF16, kind="ExternalInput").ap()
    gwh_d = nc.dram_tensor("gwh", [C, E], BF16, kind="ExternalInput").ap()
    gwl_d = nc.dram_tensor("gwl", [C, E], BF16, kind="ExternalInput").ap()
    fcw_d = nc.dram_tensor("fcw", [E, DE, H], BF16, kind="ExternalInput").ap()
    pjw_d = nc.dram_tensor("pjw", [E, 4, 128, DE], BF16, kind="ExternalInput").ap()
    fcb_d = nc.dram_tensor("fcb", [128, 64], F32, kind="ExternalInput").ap()
    pjb_d = nc.dram_tensor("pjb", [128, E], F32, kind="ExternalInput").ap()
    ngb_d = nc.dram_tensor("ngb", [16, 1], F32, kind="ExternalInput").ap()
    out_d = nc.dram_tensor("out", [C, TPC], BF16, kind="ExternalOutput").ap()

    with TileContext(nc) as tc:
        with (
            tc.tile_pool(name="wts", bufs=1) as wts,
            tc.tile_pool(name="work", bufs=2) as work,
            tc.tile_pool(name="psum", bufs=2, space="PSUM") as psum,
        ):
            # ---- tiny resident weights first (fast, unblock the gate) ----
            gwh_sb = wts.tile([128, E * E], BF16)  # [d, chunk*16+e']
            nc.sync.dma_start(
                out=gwh_sb.rearrange("p (k e) -> p k e", k=E),
                in_=gwh_d.rearrange("(k p) e -> p k e", p=128),
            )
            gwl_sb = wts.tile([128, E * E], BF16)
            nc.sync.dma_start(
                out=gwl_sb.rearrange("p (k e) -> p k e", k=E),
                in_=gwl_d.rearrange("(k p) e -> p k e", p=128),
            )
            ngb_sb = wts.tile([16, 1], F32)
            nc.sync.dma_start(out=ngb_sb, in_=ngb_d)
            fcb_sb = wts.tile([128, 64], F32)
            nc.scalar.dma_start(out=fcb_sb, in_=fcb_d)
            pjb_sb = wts.tile([128, E], F32)
            nc.scalar.dma_start(out=pjb_sb, in_=pjb_d)

            # ---- x planes: half-plane DMAs so the gate starts early ----
            xh_t = [None] * NGRP
            xl_t = [None] * NGRP

            def load_planes(g):
                t0 = g * GROUP
                xh_t[g] = work.tile(
                    [128, E * GROUP], BF16, tag="xh", bufs=2, name=f"xh_{g}"
                )
                xl_t[g] = work.tile(
                    [128, E * GROUP], BF16, tag="xl", bufs=2, name=f"xl_{g}"
                )
                for h in range(2):
                    ks = slice(h * 8, (h + 1) * 8)
                    rows = slice(h * 8 * 128, (h + 1) * 8 * 128)
                    cols = slice(h * 8 * GROUP, (h + 1) * 8 * GROUP)
                    nc.sync.dma_start(
                        out=xh_t[g][:, cols].rearrange("p (k t) -> p k t", k=8),
                        in_=xh_d[rows, t0 : t0 + GROUP].rearrange(
                            "(k p) t -> p k t", p=128
                        ),
                    )
                    nc.scalar.dma_start(
                        out=xl_t[g][:, cols].rearrange("p (k t) -> p k t", k=8),
                        in_=xl_d[rows, t0 : t0 + GROUP].rearrange(
                            "(k p) t -> p k t", p=128
                        ),
                    )

            load_planes(0)
            # fc/proj weights: per-expert slices so pass B can start early
            fcw_sb = wts.tile([128, E * H], BF16)  # [de, e*512+h]
            pjw_sb = wts.tile([128, E * 4 * DE], BF16)  # [h_chunk, (e*4+q)*128+d]
            pjw_v = pjw_sb.rearrange("p (e q d) -> p e q d", e=E, q=4)
            pjw_dv = pjw_d.rearrange("e q p d -> p e q d")
            for e in range(E):
                eng = nc.scalar if (e % 2) else nc.sync
                eng.dma_start(out=fcw_sb[:, e * H : (e + 1) * H], in_=fcw_d[e])
                eng.dma_start(out=pjw_v[:, e], in_=pjw_dv[:, e])

            for g in range(NGRP):
                t0 = g * GROUP
                if g + 1 < NGRP:
                    load_planes(g + 1)
                xh, xl = xh_t[g], xl_t[g]

                # ---- gate: 3-term bf16 accumulation, exact to ~1e-5 ----
                ps_g = psum.tile([16, GROUP], F32, tag="gate", bufs=2)
                for e in range(E):
                    sh = slice(e * GROUP, (e + 1) * GROUP)
                    sw = slice(e * E, (e + 1) * E)
                    nc.tensor.matmul(
                        ps_g, lhsT=gwh_sb[:, sw], rhs=xh[:, sh],
                        start=(e == 0), stop=False,
                    )
                    nc.tensor.matmul(
                        ps_g, lhsT=gwh_sb[:, sw], rhs=xl[:, sh],
                        start=False, stop=False,
                    )
                    nc.tensor.matmul(
                        ps_g, lhsT=gwl_sb[:, sw], rhs=xh[:, sh],
                        start=False, stop=(e == E - 1),
                    )
                # threshold: gw[e', t] = (logits0 > -gate_b[e'])
                gwt = work.tile([16, GROUP], BF16, tag="gwt", bufs=2)
                nc.vector.tensor_scalar(gwt, ps_g, ngb_sb, None, op0=ALU.is_gt)
                # flatten onto partition 0, broadcast across partitions
                gwf = work.tile([1, E * GROUP], BF16, tag="gwf", bufs=2)
                nc.sync.dma_start(
                    out=gwf.rearrange("o (e t) -> o e t", e=E), in_=gwt
                )
                gwb = work.tile([128, E * GROUP], BF16, tag="gwb", bufs=2)
                for q in range(4):
                    nc.gpsimd.partition_broadcast(
                        gwb[:, q * 4 * GROUP : (q + 1) * 4 * GROUP],
                        gwf[:, q * 4 * GROUP : (q + 1) * 4 * GROUP],
                    )

                # ---- per expert: fc -> gelu -> proj -> fused gated evac ----
                outT = None
                for e in range(E):
                    xslab = xh[:, e * GROUP : (e + 1) * GROUP]
                    h_sb = work.tile([128, 4 * GROUP], BF16, tag="h", bufs=2)
                    if split_gelu_bias:
                        for hq in range(4):
                            ps_fc = psum.tile(
                                [128, GROUP], F32, tag="fc", bufs=4
                            )
                            nc.tensor.matmul(
                                ps_fc,
                                lhsT=fcw_sb[
                                    :, e * H + hq * 128 : e * H + (hq + 1) * 128
                                ],
                                rhs=xslab,
                                start=True, stop=True,
                            )
                            nc.scalar.activation(
                                h_sb[:, hq * GROUP : (hq + 1) * GROUP],
                                ps_fc, GELU,
                                bias=fcb_sb[:, e * 4 + hq : e * 4 + hq + 1],
                                scale=1.0,
                            )
                    else:
                        for hp in range(2):
                            ps_fc = psum.tile(
                                [128, 2 * GROUP], F32, tag="fc", bufs=2
                            )
                            for hq in (2 * hp, 2 * hp + 1):
                                nc.tensor.matmul(
                                    ps_fc[:, (hq % 2) * GROUP : (hq % 2 + 1) * GROUP],
                                    lhsT=fcw_sb[
                                        :, e * H + hq * 128 : e * H + (hq + 1) * 128
                                    ],
                                    rhs=xslab,
                                    start=True, stop=True,
                                )
                            nc.scalar.activation(
                                h_sb[:, 2 * hp * GROUP : (2 * hp + 2) * GROUP],
                                ps_fc, GELU, scale=1.0,
                            )
                    ps_pj = psum.tile([128, GROUP], F32, tag="pj", bufs=2)
                    for hq in range(4):
                        nc.tensor.matmul(
                            ps_pj,
                            lhsT=pjw_sb[
                                :, (e * 4 + hq) * 128 : (e * 4 + hq + 1) * 128
                            ],
                            rhs=h_sb[:, hq * GROUP : (hq + 1) * GROUP],
                            start=(hq == 0), stop=(hq == 3),
                        )
                    if e % 4 == 0:
                        outT = work.tile([128, 4 * GROUP], BF16, tag="outT", bufs=3)
                    # out^T[d, t] = (psum + proj_b[d]) * gw[e, t]
                    nc.vector.scalar_tensor_tensor(
                        outT[:, (e % 4) * GROUP : (e % 4 + 1) * GROUP],
                        ps_pj,
                        pjb_sb[:, e : e + 1],
                        gwb[:, e * GROUP : (e + 1) * GROUP],
                        op0=ALU.add,
                        op1=ALU.mult,
                    )
                    if e % 4 == 3:
                        nc.sync.dma_start(
                            out=out_d[
                                (e - 3) * 128 : (e + 1) * 128, t0 : t0 + GROUP
                            ].rearrange("(q p) t -> p q t", p=128),
                            in_=outT.rearrange("p (q t) -> p q t", q=4),
                        )

    nc.compile()
    return nc


def _prep_inputs(x, gate_w, gate_b, fc_w, fc_b, proj_w, proj_b):
    bf16 = ml_dtypes.bfloat16
    x = np.ascontiguousarray(np.asarray(x, dtype=np.float32)).reshape(NTOK, C)
    gate_w = np.asarray(gate_w, dtype=np.float32)
    gate_b = np.asarray(gate_b, dtype=np.float32)
    fc_w = np.asarray(fc_w, dtype=np.float32)
    fc_b = np.asarray(fc_b, dtype=np.float32)
    proj_w = np.asarray(proj_w, dtype=np.float32)
    proj_b = np.asarray(proj_b, dtype=np.float32)

    # permuted channel order: c' = e*128 + d  ->  orig c = 16*d + e
    cp = np.arange(C)
    orig = 16 * (cp % DE) + cp // DE
    xp = x[:, orig]  # (e, d)-channel-major
    xh = xp.astype(bf16)
    xl = (xp - xh.astype(np.float32)).astype(bf16)
    # host-side transpose: ship channel-major planes so the device loads
    # expert slabs with plain contiguous-row DMAs (no on-device transpose)
    xh = np.ascontiguousarray(xh.reshape(NCORES, TPC, C).transpose(0, 2, 1))
    xl = np.ascontiguousarray(xl.reshape(NCORES, TPC, C).transpose(0, 2, 1))
    gwp = gate_w[:, orig].T  # [C', E]
    gwh = gwp.astype(bf16)
    gwl = (gwp - gwh.astype(np.float32)).astype(bf16)
    fcw = np.ascontiguousarray(fc_w.transpose(0, 2, 1)).astype(bf16)  # [E, DE, H]
    pjw = np.ascontiguousarray(
        proj_w.transpose(0, 2, 1).reshape(E, 4, 128, DE)
    ).astype(bf16)  # [E, q, h_chunk, d]
    fcb = np.ascontiguousarray(
        fc_b.reshape(E, 4, 128).transpose(2, 0, 1).reshape(128, E * 4)
    )
    pjb = np.ascontiguousarray(proj_b.T)  # [DE, E]
    ngb = np.ascontiguousarray((-gate_b).reshape(16, 1))

    shared = {
        "gwh": np.ascontiguousarray(gwh),
        "gwl": np.ascontiguousarray(gwl),
        "fcw": fcw,
        "pjw": pjw,
        "fcb": fcb,
        "pjb": pjb,
        "ngb": ngb,
    }
    in_maps = [
        {"xh": xh[i], "xl": xl[i], **shared}
        for i in range(NCORES)
    ]
    return in_maps, bool(np.any(fc_b))


def kernel(x, gate_w, gate_b, fc_w, fc_b, proj_w, proj_b, _trace=False, _tmpdir=None):
    in_maps, split_bias = _prep_inputs(
        x, gate_w, gate_b, fc_w, fc_b, proj_w, proj_b
    )
    key = ("nc", split_bias)
    if key not in _CACHE:
        _CACHE[key] = _build(split_bias)
    nc = _CACHE[key]
    res = run_bass_kernel_spmd(
        nc,
        in_maps,
        core_ids=list(range(NCORES)),
        trace=_trace,
        tmpdir=_tmpdir,
    )
    # per-core output is channel-major bf16 [C, TPC]; transpose + upcast
    out = np.concatenate(
        [
            np.ascontiguousarray(res.results[i]["out"].T.astype(np.float32))
            for i in range(NCORES)
        ],
        axis=0,
    )
    out = out.reshape(B, T, C)
    if _trace:
        _CACHE["last_result"] = res
    return out


# revision 18
# speedup vs baseline: 1.2999x; 1.0397x over previous
"""DynamicSparseMoE Trainium2 kernel (v3: DMA-transposed bf16 planes).

Math (per token t):
  logits[e'] = x[t] . gate_w[e'] + gate_b[e']        (C=2048 contraction)
  gw[e']     = 1.0 if logits[e'] > 0 else 0.0
  expert e input: xe[d] = x[t, 16*d + e]  (d=0..127; expert idx fastest in channel)
  h  = gelu(fc_w[e] @ xe + fc_b[e])                   (H=512)
  oe = proj_w[e] @ h + proj_b[e]                      (DE=128)
  out[t, 128*e + d] = gw[e] * oe[d]                   (expert-major output channels)

Strategy: data-parallel over the 16384 tokens across 8 NeuronCores (2048
tokens/core).  The host pre-permutes x to (expert, d)-channel-major order and
splits it into bf16 hi/lo planes (x = xh + xl exactly to ~2^-17).  Per
512-token group:
  - one DMA-XBAR transpose per plane loads all 16 expert tiles [d, tok]
    directly into SBUF (no PE transposes, no PSUM evacuations).
  - gate: 3-term bf16 accumulation xh@gh + xl@gh + xh@gl into one PSUM bank
    (~1e-5 logit accuracy, exact thresholding); DVE threshold vs -gate_b;
    flatten to partition 0 (DMA) and GPSIMD-broadcast to [128, 16*tok].
  - per expert: fc as bf16 matmuls (weights stationary, N=512) on the hi
    plane; gelu fused on ACT (batched over 2 PSUM banks when fc_b == 0);
    proj as bf16 matmuls accumulating K=512; single fused DVE
    scalar_tensor_tensor evacuation out^T = (psum + proj_b) * gw_bcast, bf16.
  Output is CHANNEL-major ([C, tokens]) bf16 in DRAM; the host transposes and
  upcasts while unsharding.
"""

import sys

for _p in ("/opt/trn_rl_repo", "/root/.axon_site"):
    if _p not in sys.path:
        sys.path.insert(0, _p)

import ml_dtypes
import numpy as np

import concourse.mybir as mybir
from concourse import bacc
from concourse.bass_utils import run_bass_kernel_spmd
from concourse.tile import TileContext


B, T, C, E = 8, 2048, 2048, 16
DE = C // E  # 128
H = 4 * DE  # 512
NCORES = 8
NTOK = B * T  # 16384
TPC = NTOK // NCORES  # tokens per core: 2048
GROUP = 512  # tokens per group
NGRP = TPC // GROUP  # 4 groups per core

F32 = mybir.dt.float32
BF16 = mybir.dt.bfloat16
AF = mybir.ActivationFunctionType
ALU = mybir.AluOpType
GELU = AF.Gelu

_CACHE = {}


def _build(split_gelu_bias: bool):
    nc = bacc.Bacc(trn_type="TRN2", num_devices=NCORES)

    xh_d = nc.dram_tensor("xh", [C, TPC], BF16, kind="ExternalInput").ap()
    xl_d = nc.dram_tensor("xl", [C, TPC], BF16, kind="ExternalInput").ap()
    gwh_d = nc.dram_tensor("gwh", [C, E], BF16, kind="ExternalInput").ap()
    gwl_d = nc.dram_tensor("gwl", [C, E], BF16, kind="ExternalInput").ap()
    fcw_d = nc.dram_tensor("fcw", [E, DE, H], BF16, kind="ExternalInput").ap()
    pjw_d = nc.dram_tensor("pjw", [E, 4, 128, DE], BF16, kind="ExternalInput").ap()
    fcb_d = nc.dram_tensor("fcb", [128, 64], F32, kind="ExternalInput").ap()
    pjb_d = nc.dram_tensor("pjb", [128, E], F32, kind="ExternalInput").ap()
    ngb_d = nc.dram_tensor("ngb", [16, 1], F32, kind="ExternalInput").ap()
    out_d = nc.dram_tensor("out", [C, TPC], BF16, kind="ExternalOutput").ap()

    with TileContext(nc) as tc:
        with (
            tc.tile_pool(name="wts", bufs=1) as wts,
            tc.tile_pool(name="work", bufs=2) as work,
            tc.tile_pool(name="psum", bufs=2, space="PSUM") as psum,
        ):
            # ---- tiny resident weights first (fast, unblock the gate) ----
            gwh_sb = wts.tile([128, E * E], BF16)  # [d, chunk*16+e']
            nc.sync.dma_start(
                out=gwh_sb.rearrange("p (k e) -> p k e", k=E),
                in_=gwh_d.rearrange("(k p) e -> p k e", p=128),
            )
            gwl_sb = wts.tile([128, E * E], BF16)
            nc.sync.dma_start(
                out=gwl_sb.rearrange("p (k e) -> p k e", k=E),
                in_=gwl_d.rearrange("(k p) e -> p k e", p=128),
            )
            ngb_sb = wts.tile([16, 1], F32)
            nc.sync.dma_start(out=ngb_sb, in_=ngb_d)
            fcb_sb = wts.tile([128, 64], F32)
            nc.scalar.dma_start(out=fcb_sb, in_=fcb_d)
            pjb_sb = wts.tile([128, E], F32)
            nc.scalar.dma_start(out=pjb_sb, in_=pjb_d)

            # ---- x planes: separate half tiles (fine-grained deps) ----
            xh_t = [None] * NGRP
            xl_t = [None] * NGRP

            def load_planes(g):
                t0 = g * GROUP
                xh_t[g] = []
                xl_t[g] = []
                for h in range(2):
                    rows = slice(h * 8 * 128, (h + 1) * 8 * 128)
                    th = work.tile(
                        [128, 8 * GROUP], BF16, tag=f"xh{h}", bufs=3,
                        name=f"xh_{g}_{h}",
                    )
                    nc.sync.dma_start(
                        out=th.rearrange("p (k t) -> p k t", k=8),
                        in_=xh_d[rows, t0 : t0 + GROUP].rearrange(
                            "(k p) t -> p k t", p=128
                        ),
                    )
                    xh_t[g].append(th)
                    tl = work.tile(
                        [128, 8 * GROUP], BF16, tag=f"xl{h}", bufs=2,
                        name=f"xl_{g}_{h}",
                    )
                    nc.scalar.dma_start(
                        out=tl.rearrange("p (k t) -> p k t", k=8),
                        in_=xl_d[rows, t0 : t0 + GROUP].rearrange(
                            "(k p) t -> p k t", p=128
                        ),
                    )
                    xl_t[g].append(tl)

            def slab(planes, e):
                return planes[e // 8][:, (e % 8) * GROUP : (e % 8 + 1) * GROUP]

            load_planes(0)
            # fc/proj weights: per-expert slices so pass B can start early
            fcw_sb = wts.tile([128, E * H], BF16)  # [de, e*512+h]
            pjw_sb = wts.tile([128, E * 4 * DE], BF16)  # [h_chunk, (e*4+q)*128+d]
            pjw_v = pjw_sb.rearrange("p (e q d) -> p e q d", e=E, q=4)
            pjw_dv = pjw_d.rearrange("e q p d -> p e q d")
            for e in range(E):
                eng = nc.scalar if (e % 2) else nc.sync
                eng.dma_start(out=fcw_sb[:, e * H : (e + 1) * H], in_=fcw_d[e])
                eng.dma_start(out=pjw_v[:, e], in_=pjw_dv[:, e])
            load_planes(1)

            for g in range(NGRP):
                t0 = g * GROUP
                if g + 2 < NGRP:
                    load_planes(g + 2)
                xh, xl = xh_t[g], xl_t[g]

                # ---- gate: 3-term bf16 accumulation, exact to ~1e-5 ----
                ps_g = psum.tile([16, GROUP], F32, tag="gate", bufs=2)
                for e in range(E):
                    sw = slice(e * E, (e + 1) * E)
                    nc.tensor.matmul(
                        ps_g, lhsT=gwh_sb[:, sw], rhs=slab(xh, e),
                        start=(e == 0), stop=False,
                    )
                    nc.tensor.matmul(
                        ps_g, lhsT=gwh_sb[:, sw], rhs=slab(xl, e),
                        start=False, stop=False,
                    )
                    nc.tensor.matmul(
                        ps_g, lhsT=gwl_sb[:, sw], rhs=slab(xh, e),
                        start=False, stop=(e == E - 1),
                    )
                # threshold: gw[e', t] = (logits0 > -gate_b[e'])
                gwt = work.tile([16, GROUP], BF16, tag="gwt", bufs=2)
                nc.vector.tensor_scalar(gwt, ps_g, ngb_sb, None, op0=ALU.is_gt)
                # flatten onto partition 0, broadcast across partitions
                gwf = work.tile([1, E * GROUP], BF16, tag="gwf", bufs=2)
                nc.sync.dma_start(
                    out=gwf.rearrange("o (e t) -> o e t", e=E), in_=gwt
                )
                gwb = work.tile([128, E * GROUP], BF16, tag="gwb", bufs=2)
                for q in range(4):
                    nc.gpsimd.partition_broadcast(
                        gwb[:, q * 4 * GROUP : (q + 1) * 4 * GROUP],
                        gwf[:, q * 4 * GROUP : (q + 1) * 4 * GROUP],
                    )

                # ---- per expert: fc -> gelu -> proj -> fused gated evac ----
                outT = None
                for e in range(E):
                    xslab = slab(xh, e)
                    h_sb = work.tile([128, 4 * GROUP], BF16, tag="h", bufs=2)
                    if split_gelu_bias:
                        for hq in range(4):
                            ps_fc = psum.tile(
                                [128, GROUP], F32, tag="fc", bufs=4
                            )
                            nc.tensor.matmul(
                                ps_fc,
                                lhsT=fcw_sb[
                                    :, e * H + hq * 128 : e * H + (hq + 1) * 128
                                ],
                                rhs=xslab,
                                start=True, stop=True,
                            )
                            nc.scalar.activation(
                                h_sb[:, hq * GROUP : (hq + 1) * GROUP],
                                ps_fc, GELU,
                                bias=fcb_sb[:, e * 4 + hq : e * 4 + hq + 1],
                                scale=1.0,
                            )
                    else:
                        for hp in range(2):
                            ps_fc = psum.tile(
                                [128, 2 * GROUP], F32, tag="fc", bufs=2
                            )
                            for hq in (2 * hp, 2 * hp + 1):
                                nc.tensor.matmul(
                                    ps_fc[:, (hq % 2) * GROUP : (hq % 2 + 1) * GROUP],
                                    lhsT=fcw_sb[
                                        :, e * H + hq * 128 : e * H + (hq + 1) * 128
                                    ],
                                    rhs=xslab,
                                    start=True, stop=True,
                                )
                            nc.scalar.activation(
                                h_sb[:, 2 * hp * GROUP : (2 * hp + 2) * GROUP],
                                ps_fc, GELU, scale=1.0,
                            )
                    ps_pj = psum.tile([128, GROUP], F32, tag="pj", bufs=2)
                    for hq in range(4):
                        nc.tensor.matmul(
                            ps_pj,
                            lhsT=pjw_sb[
                                :, (e * 4 + hq) * 128 : (e * 4 + hq + 1) * 128
                            ],
                            rhs=h_sb[:, hq * GROUP : (hq + 1) * GROUP],
                            start=(hq == 0), stop=(hq == 3),
                        )
                    if e % 4 == 0:
                        outT = work.tile([128, 4 * GROUP], BF16, tag="outT", bufs=2)
                    # out^T[d, t] = (psum + proj_b[d]) * gw[e, t]
                    nc.vector.scalar_tensor_tensor(
                        outT[:, (e % 4) * GROUP : (e % 4 + 1) * GROUP],
                        ps_pj,
                        pjb_sb[:, e : e + 1],
                        gwb[:, e * GROUP : (e + 1) * GROUP],
                        op0=ALU.add,
                        op1=ALU.mult,
                    )
                    if e % 4 == 3:
                        nc.sync.dma_start(
                            out=out_d[
                                (e - 3) * 128 : (e + 1) * 128, t0 : t0 + GROUP
                            ].rearrange("(q p) t -> p q t", p=128),
                            in_=outT.rearrange("p (q t) -> p q t", q=4),
                        )

    nc.compile()
    return nc


def _prep_inputs(x, gate_w, gate_b, fc_w, fc_b, proj_w, proj_b):
    bf16 = ml_dtypes.bfloat16
    x = np.ascontiguousarray(np.asarray(x, dtype=np.float32)).reshape(NTOK, C)
    gate_w = np.asarray(gate_w, dtype=np.float32)
    gate_b = np.asarray(gate_b, dtype=np.float32)
    fc_w = np.asarray(fc_w, dtype=np.float32)
    fc_b = np.asarray(fc_b, dtype=np.float32)
    proj_w = np.asarray(proj_w, dtype=np.float32)
    proj_b = np.asarray(proj_b, dtype=np.float32)

    # permuted channel order: c' = e*128 + d  ->  orig c = 16*d + e
    cp = np.arange(C)
    orig = 16 * (cp % DE) + cp // DE
    xp = x[:, orig]  # (e, d)-channel-major
    xh = xp.astype(bf16)
    xl = (xp - xh.astype(np.float32)).astype(bf16)
    # host-side transpose: ship channel-major planes so the device loads
    # expert slabs with plain contiguous-row DMAs (no on-device transpose)
    xh = np.ascontiguousarray(xh.reshape(NCORES, TPC, C).transpose(0, 2, 1))
    xl = np.ascontiguousarray(xl.reshape(NCORES, TPC, C).transpose(0, 2, 1))
    gwp = gate_w[:, orig].T  # [C', E]
    gwh = gwp.astype(bf16)
    gwl = (gwp - gwh.astype(np.float32)).astype(bf16)
    fcw = np.ascontiguousarray(fc_w.transpose(0, 2, 1)).astype(bf16)  # [E, DE, H]
    pjw = np.ascontiguousarray(
        proj_w.transpose(0, 2, 1).reshape(E, 4, 128, DE)
    ).astype(bf16)  # [E, q, h_chunk, d]
    fcb = np.ascontiguousarray(
        fc_b.reshape(E, 4, 128).transpose(2, 0, 1).reshape(128, E * 4)
    )
    pjb = np.ascontiguousarray(proj_b.T)  # [DE, E]
    ngb = np.ascontiguousarray((-gate_b).reshape(16, 1))

    shared = {
        "gwh": np.ascontiguousarray(gwh),
        "gwl": np.ascontiguousarray(gwl),
        "fcw": fcw,
        "pjw": pjw,
        "fcb": fcb,
        "pjb": pjb,
        "ngb": ngb,
    }
    in_maps = [
        {"xh": xh[i], "xl": xl[i], **shared}
        for i in range(NCORES)
    ]
    return in_maps, bool(np.any(fc_b))


def kernel(x, gate_w, gate_b, fc_w, fc_b, proj_w, proj_b, _trace=False, _tmpdir=None):
    in_maps, split_bias = _prep_inputs(
        x, gate_w, gate_b, fc_w, fc_b, proj_w, proj_b
    )
    key = ("nc", split_bias)
    if key not in _CACHE:
        _CACHE[key] = _build(split_bias)
    nc = _CACHE[key]
    res = run_bass_kernel_spmd(
        nc,
        in_maps,
        core_ids=list(range(NCORES)),
        trace=_trace,
        tmpdir=_tmpdir,
    )
    # per-core output is channel-major bf16 [C, TPC]; transpose + upcast
    out = np.concatenate(
        [
            np.ascontiguousarray(res.results[i]["out"].T.astype(np.float32))
            for i in range(NCORES)
        ],
        axis=0,
    )
    out = out.reshape(B, T, C)
    if _trace:
        _CACHE["last_result"] = res
    return out
